# revision 14
# baseline (speedup 1.0000x reference)
"""Trainium2 Bass kernel for the differentiable Gaussian renderer.

Strategy (v3)
-------------
Host (numpy, 512 Gaussians, negligible):
  - mirror the reference projection exactly: quat->rot, 3D cov, camera
    transform, 2D cov (+eps fix), conic, visibility, back-to-front sort.
  - split the image into 96 tiles of 8 rows x 64 cols (512 px).  Per tile,
    cull Gaussians whose max alpha over the tile is < ALPHA_CUT (exact
    quadratic max over the tile rectangle).  Per-tile recentered (x', y')
    coordinates keep fp32/f32r cancellation error small.
  - PARTITION-PACK two tiles into one 128-row "unit" (globally fold-pair
    the 96 tiles by row count; 48 pairs; 6 pairs per core).  ACT cost on
    TRN2 scales only with the free (pixel) dim, so packing two tiles onto
    disjoint partition ranges of the same [128, 512px] tensors halves the
    scalar-engine work — the kernel bottleneck.
  - TELESCOPED compositing: alpha_i*T_i = T_i - T_{i+1}, so
    img = sum_i (c_i - c_{i-1}) * T_i with T_i = exp(S_i),
    S_i = sum_{j<i} ln(1 - alpha_j) (strict cumsum within each tile
    segment).  The background becomes a final row with color bg and
    alpha irrelevant; no exp(E+S) in-place PSUM accumulation is needed.

Device (8 cores SPMD, 6 units each = 12 tiles):
  Gaussians on partitions (depth order, two tile segments per unit),
  512 tile-local pixels on the free dim (pixel features are identical
  for every tile after recentering).  Per unit:
  - PE : E = gc^T(6x128) @ pf(6x512)                  -> PSUM
  - ACT: A = exp(E)                                   -> SBUF  (per duo)
  - ACT: L = ln(1 - A)                                -> SBUF  (per duo)
  - PE : S = blocktriu_u^T @ L    (overwrites E bank) -> PSUM
  - ACT: T = exp(S)                                   -> SBUF  (per duo)
  - PE : img[6u:6u+6, px] = dcol_u^T @ T              -> one PSUM bank
  One [6U,512] copy + one output DMA at the end.  ACT instructions are
  emitted in an order that keeps the strict-FIFO scalar queue busy.
"""

import numpy as np

import concourse.bacc as bacc
import concourse.tile as _tile_mod
from concourse import hw_specs as _hw_specs, mybir
from concourse.bass_utils import run_bass_kernel_spmd


def _mono_act_tables(module_arch):
    """Blank every activation-table set except the one holding BOTH exp and
    ln, so the table loader never thrashes between Exp and Ln tables."""
    tables = _hw_specs.get_activation_tables(module_arch)
    keep = "natural_log_exp_and_others"
    if keep in tables:
        return {n: (s if n == keep else set()) for n, s in tables.items()}
    return tables


bacc.get_activation_tables = _mono_act_tables

H, W = 192, 256
NEAR, FAR = 0.1, 100.0
N = 512
N_CORES = 8
TILE_R, TILE_C = 16, 32
TILE_PX = TILE_R * TILE_C          # 512
N_TILES_Y = H // TILE_R            # 12
N_TILES_X = W // TILE_C            # 8
N_TILES = N_TILES_Y * N_TILES_X    # 96
ALPHA_CUT = 2e-2                   # cull-only rel err ~2.6e-3 (budget 2e-2)
F32 = mybir.dt.float32
F32R = mybir.dt.float32r

_TileContext = _tile_mod.TileContext


# ---------------------------------------------------------------- host math

def _project_and_sort(positions, scales, rotations, opacities, colors,
                      view_matrix, fov_x):
    f8 = np.float64
    pos = positions.astype(f8)
    scl = scales.astype(f8)
    rot = rotations.astype(f8)
    opa = opacities.astype(f8)
    col = colors.astype(f8)
    vm = view_matrix.astype(f8)

    qn = rot / np.linalg.norm(rot, axis=-1, keepdims=True)
    w, x, y, z = qn[:, 0], qn[:, 1], qn[:, 2], qn[:, 3]
    R = np.stack([
        1 - 2 * (y * y + z * z), 2 * (x * y - w * z), 2 * (x * z + w * y),
        2 * (x * y + w * z), 1 - 2 * (x * x + z * z), 2 * (y * z - w * x),
        2 * (x * z - w * y), 2 * (y * z + w * x), 1 - 2 * (x * x + y * y),
    ], axis=-1).reshape(-1, 3, 3)
    cov3d = np.einsum('nij,nj,nkj->nik', R, scl ** 2, R)

    fx = W / (2.0 * np.tan(np.deg2rad(float(fov_x)) / 2.0))
    Wr = vm[:3, :3]
    t = vm[:3, 3]
    cam = pos @ Wr.T + t[None, :]
    depths = cam[:, 2]
    zs = np.maximum(depths, NEAR)
    X, Y = cam[:, 0], cam[:, 1]
    mx = fx * X / zs + W / 2.0
    my = H / 2.0 - fx * Y / zs
    zero = np.zeros_like(zs)
    J = np.stack([
        np.stack([fx / zs, zero, -fx * X / zs ** 2], axis=-1),
        np.stack([zero, fx / zs, -fx * Y / zs ** 2], axis=-1),
    ], axis=1)
    T2 = np.einsum('nij,jk->nik', J, Wr)
    cov2d = np.einsum('nij,njk,nlk->nil', T2, cov3d, T2)
    cov2d = 0.5 * (cov2d + np.swapaxes(cov2d, 1, 2))

    a, b, c = cov2d[:, 0, 0], cov2d[:, 0, 1], cov2d[:, 1, 1]
    mean_e = 0.5 * (a + c)
    disc = np.sqrt(np.maximum(0.25 * (a - c) ** 2 + b ** 2, 0.0))
    min_eig = mean_e - disc
    eps = np.where(min_eig <= 0, np.abs(min_eig) + 1e-6, 0.0)
    a = a + eps
    c = c + eps
    max_eig = mean_e + eps + disc
    radii = np.ceil(3.0 * np.sqrt(np.maximum(max_eig, 1e-6)))

    visible = (depths > NEAR) & (depths < FAR) & (radii > 0)
    # float32 keys + stable sort reproduce jnp.argsort's order exactly
    key = np.where(visible, -depths.astype(np.float32), np.inf).astype(np.float32)
    order = np.argsort(key, kind='stable')

    a_s, b_s, c_s = a[order], b[order], c[order]
    det = np.maximum(a_s * c_s - b_s * b_s, 1e-12)
    vis = visible[order]
    ca = np.where(vis, c_s / det, 0.0)
    cb = np.where(vis, -b_s / det, 0.0)
    cc = np.where(vis, a_s / det, 0.0)
    op = 1.0 / (1.0 + np.exp(-opa[order, 0]))
    return dict(
        ca=ca, cb=cb, cc=cc,
        mx=np.where(vis, mx[order], 0.0), my=np.where(vis, my[order], 0.0),
        L0=np.where(vis, np.log(np.maximum(op, 1e-300)), -100.0),
        op=op, col=np.where(vis[:, None], col[order], 0.0), vis=vis,
    )


def _tile_max_E(g, yc, ye, xc, xe):
    """Per-Gaussian max of E over the pixel rectangle [xc+-xe] x [yc+-ye]."""
    ca, cb, cc = g['ca'], g['cb'], g['cc']
    mx, my, L0 = g['mx'], g['my'], g['L0']
    best = np.where((my >= yc - ye) & (my <= yc + ye)
                    & (mx >= xc - xe) & (mx <= xc + xe), L0, -np.inf)
    safe_ca = np.where(ca > 0, ca, 1.0)
    safe_cc = np.where(cc > 0, cc, 1.0)
    for yv in (yc - ye, yc + ye):
        dy = yv - my
        xstar = np.clip(np.where(ca > 0, mx - cb * dy / safe_ca, mx),
                        xc - xe, xc + xe)
        for xv in (xstar, np.full_like(xstar, xc - xe),
                   np.full_like(xstar, xc + xe)):
            dx = xv - mx
            E = -0.5 * (ca * dx * dx + 2 * cb * dx * dy + cc * dy * dy) + L0
            best = np.maximum(best, E)
    for xv in (xc - xe, xc + xe):
        dx = xv - mx
        ystar = np.clip(np.where(cc > 0, my - cb * dx / safe_cc, my),
                        yc - ye, yc + ye)
        dy = ystar - my
        E = -0.5 * (ca * dx * dx + 2 * cb * dx * dy + cc * dy * dy) + L0
        best = np.maximum(best, E)
    return np.where(g['vis'], best, -np.inf)


def _pixel_features():
    """[6, 512] recentered tile pixel features, row-major within the tile."""
    xs = np.arange(TILE_C, dtype=np.float64) - (TILE_C - 1) / 2.0
    ys = np.arange(TILE_R, dtype=np.float64) - (TILE_R - 1) / 2.0
    yy, xx = np.meshgrid(ys, xs, indexing='ij')
    xx = xx.ravel()
    yy = yy.ravel()
    feats = np.stack([xx * xx, yy * yy, xx * yy, xx, yy,
                      np.ones_like(xx)], axis=0)
    return feats.astype(np.float32)


def _segment_data(g, keep, yc, xc, bg_color):
    """E-coefficients [6, m], delta-colors [m, 3] for one tile segment.

    Rows = culled Gaussians in depth order, then one background row whose
    color closes the telescoped sum (its alpha is never used).
    """
    ca, cb, cc = g['ca'][keep], g['cb'][keep], g['cc'][keep]
    mxp = g['mx'][keep] - xc
    myp = g['my'][keep] - yc
    L0 = g['L0'][keep]
    n = len(keep)
    m = n + 1
    gc = np.empty((6, m), np.float64)
    gc[0, :n] = -0.5 * ca
    gc[1, :n] = -0.5 * cc
    gc[2, :n] = -cb
    gc[3, :n] = ca * mxp + cb * myp
    gc[4, :n] = cc * myp + cb * mxp
    gc[5, :n] = -0.5 * (ca * mxp ** 2 + 2 * cb * mxp * myp
                        + cc * myp ** 2) + L0
    gc[:, n] = 0.0
    gc[5, n] = -100.0                     # bg row: alpha ~ 0 (unused)
    cols = np.empty((m, 3), np.float64)
    cols[:n] = g['col'][keep]
    cols[n] = bg_color
    dcol = np.empty_like(cols)
    dcol[0] = cols[0]
    dcol[1:] = cols[1:] - cols[:-1]       # telescoped colors
    return gc.astype(np.float32), dcol.astype(np.float32)


# ------------------------------------------------------------- device program

_NC_CACHE = {}


def _stage_widths(n_units):
    """Pipeline stage widths: pairs, then two single-unit drain stages."""
    widths = []
    rem = int(n_units)
    while rem > 2:
        widths.append(2)
        rem -= 2
    widths.extend([1] * rem)
    return widths


def _build_nc(n_units):
    """Device program for n_units partition-packed units per core."""
    U = int(n_units)
    widths = _stage_widths(U)
    n_stages = len(widths)
    offs = np.concatenate([[0], np.cumsum(widths)]).astype(int)

    nc = bacc.Bacc()
    # gp = pixel features || per-unit E-coefficient blocks (6 partitions)
    gp_d = nc.dram_tensor("gp", [6, TILE_PX + U * 128], F32R,
                          kind="ExternalInput")
    # tcd = per-unit block-triu masks || per-unit delta-colors
    tcd_d = nc.dram_tensor("tcd", [128, U * 128 + U * 18], F32R,
                           kind="ExternalInput")
    img_d = nc.dram_tensor("img", [18, U * TILE_PX], F32,
                           kind="ExternalOutput")

    EXP = mybir.ActivationFunctionType.Exp
    LN = mybir.ActivationFunctionType.Ln

    with _TileContext(nc) as tc:
        with (
            tc.tile_pool(name="consts", bufs=1) as consts,
            tc.tile_pool(name="abuf", bufs=2) as apool,
            tc.tile_pool(name="lbuf", bufs=2) as lpool,
            tc.tile_pool(name="tbuf", bufs=2) as tpool,
            tc.tile_pool(name="obuf", bufs=3) as obuf,
            tc.tile_pool(name="espsum", bufs=min(n_stages, 4),
                         space="PSUM") as epool,
        ):
            gp_sb = consts.tile([6, TILE_PX + U * 128], F32R)
            # first piece only carries what gates the first E matmuls;
            # ScalarE's HWDGE queue is free before the table load, so it
            # issues ~1us earlier than SP (which still has preamble work)
            w0 = TILE_PX + widths[0] * 128
            nc.scalar.dma_start(out=gp_sb[:, 0:w0], in_=gp_d[:, 0:w0])
            nc.sync.dma_start(out=gp_sb[:, w0:], in_=gp_d[:, w0:])
            tcd_sb = consts.tile([128, U * 128 + U * 18], F32R)
            nc.sync.dma_start(out=tcd_sb, in_=tcd_d[:])

            pf = gp_sb[:, 0:TILE_PX]
            gc_sb = gp_sb[:, TILE_PX:].rearrange("p (u g) -> p u g", g=128)
            triu_sb = tcd_sb[:, 0:U * 128].rearrange(
                "p (u g) -> p u g", g=128)
            dcol_sb = tcd_sb[:, U * 128:].rearrange(
                "p (u s) -> p u s", s=18)

            es_tiles = {}

            def emit_e(s):
                es = epool.tile([128, widths[s], TILE_PX], F32, tag="es")
                es_tiles[s] = es
                for j in range(widths[s]):
                    nc.tensor.matmul(es[:, j, :], gc_sb[:, offs[s] + j, :],
                                     pf, start=True, stop=True)

            def act(pool, dt, func, src, s, **kw):
                t = pool.tile([128, widths[s], TILE_PX], dt, tag=pool.name)
                nc.scalar.activation(
                    t.rearrange("p a b -> p (a b)"),
                    src.rearrange("p a b -> p (a b)"), func, **kw)
                return t

            # software pipeline over stages; ScalarE queue is strict FIFO;
            # intended ACT order: exp0 ln0 exp1 ln1 T0 exp2 ln2 T1 ...
            a_t = {}
            l_t = {}
            emit_e(0)
            a_t[0] = act(apool, F32, EXP, es_tiles[0], 0)
            l_t[0] = act(lpool, F32R, LN, a_t[0], 0, bias=1.0, scale=-1.0)
            for s in range(n_stages):
                es = es_tiles[s]
                # S = blocktriu^T @ L overwrites the E bank (E fully
                # consumed by exp); separate accumulation group.
                for j in range(widths[s]):
                    nc.tensor.matmul(es[:, j, :],
                                     triu_sb[:, offs[s] + j, :],
                                     l_t[s][:, j, :], start=True, stop=True)
                if s + 1 < n_stages:
                    emit_e(s + 1)
                    a_t[s + 1] = act(apool, F32, EXP, es_tiles[s + 1], s + 1)
                    l_t[s + 1] = act(lpool, F32R, LN, a_t[s + 1], s + 1,
                                     bias=1.0, scale=-1.0)
                t_t = act(tpool, F32R, EXP, es, s)
                for j in range(widths[s]):
                    u = offs[s] + j
                    # the stage's own E/S bank is dead after the T exp read;
                    # park the unit's 18-row image in its rows 0:18 (PE can
                    # only write PSUM at 32-aligned partition bases)
                    nc.tensor.matmul(es[0:18, j, :],
                                     dcol_sb[:, u, :], t_t[:, j, :],
                                     start=True, stop=True)
                ob = obuf.tile([18, widths[s], TILE_PX], F32, tag="ob")
                if s == n_stages - 1:
                    # ScalarE is idle after the final T exp; DVE may still
                    # be draining the previous stage's wider copy.  Its
                    # HWDGE queue is also free, unlike SP which may still
                    # be issuing the previous stage's output DMA.
                    nc.scalar.copy(ob.rearrange("p a b -> p (a b)"),
                                   es[0:18, :, :].rearrange("p a b -> p (a b)"))
                    nc.scalar.dma_start(
                        out=img_d[:, offs[s] * TILE_PX:offs[s + 1] * TILE_PX],
                        in_=ob.rearrange("p a b -> p (a b)"))
                else:
                    nc.vector.tensor_copy(ob, es[0:18, :, :])
                    nc.sync.dma_start(
                        out=img_d[:, offs[s] * TILE_PX:offs[s + 1] * TILE_PX],
                        in_=ob.rearrange("p a b -> p (a b)"))
    nc.finalize()
    return nc


def _get_nc(n_units):
    key = int(n_units)
    if key not in _NC_CACHE:
        _NC_CACHE[key] = _build_nc(key)
    return _NC_CACHE[key]


# ----------------------------------------------------------------- entrypoint

def kernel(positions, scales, rotations, opacities, colors, view_matrix,
           background, fov_x):
    g = _project_and_sort(positions, scales, rotations, opacities, colors,
                          view_matrix, fov_x)
    assert g['op'][g['vis']].max() < 0.985, "alpha clip at 0.99 would activate"
    bg = np.asarray(background, np.float64)

    cut = float(np.log(ALPHA_CUT))
    xe = (TILE_C - 1) / 2.0
    ye = (TILE_R - 1) / 2.0

    tiles = []                       # (m, yi, xi, keep); m = rows incl. bg
    for yi in range(N_TILES_Y):
        yc = yi * TILE_R + ye
        for xi in range(N_TILES_X):
            xc = xi * TILE_C + xe
            keep = np.nonzero(_tile_max_E(g, yc, ye, xc, xe) >= cut)[0]
            assert len(keep) + 1 <= 128, "tile exceeds one partition unit"
            tiles.append((len(keep) + 1, yi, xi, keep))

    # First-fit-decreasing: pack tiles into 128-row units, <=3 tiles each.
    tiles.sort(key=lambda t: -t[0])
    units = []                       # list of lists of tiles
    for t in tiles:
        for u in units:
            if sum(x[0] for x in u) + t[0] <= 128 and len(u) < 6:
                u.append(t)
                break
        else:
            units.append([t])
    # Deal units round-robin to cores; every unit costs the same on device.
    core_units = [units[c::N_CORES] for c in range(N_CORES)]
    U = max(len(cu) for cu in core_units)

    pf = _pixel_features()

    in_maps = []
    for c in range(N_CORES):
        gc_dev = np.zeros((6, U, 128), np.float32)
        gc_dev[5] = -100.0                           # padding: alpha ~ 0
        triu_dev = np.zeros((128, U, 128), np.float32)
        dcol_dev = np.zeros((128, U, 18), np.float32)
        for u, unit in enumerate(core_units[c]):
            r = 0
            for a, tile in enumerate(unit):
                m, yi, xi, keep = tile
                yc = yi * TILE_R + ye
                xc = xi * TILE_C + xe
                gc_t, dcol_t = _segment_data(g, keep, yc, xc, bg)
                gc_dev[:, u, r:r + m] = gc_t
                dcol_dev[r:r + m, u, 3 * a:3 * a + 3] = dcol_t
                triu_dev[r:r + m, u, r:r + m] = np.triu(
                    np.ones((m, m), np.float32), 1)
                r += m
        gp = np.concatenate(
            [pf, gc_dev.reshape(6, U * 128)], axis=1)
        tcd = np.concatenate(
            [triu_dev.reshape(128, U * 128),
             dcol_dev.reshape(128, U * 18)], axis=1)
        in_maps.append(dict(gp=np.ascontiguousarray(gp),
                            tcd=np.ascontiguousarray(tcd)))

    res = run_bass_kernel_spmd(_get_nc(U), in_maps,
                               core_ids=list(range(N_CORES)))

    image = np.empty((H, W, 3), np.float32)
    for c in range(N_CORES):
        img = res.results[c]["img"].reshape(18, -1, TILE_PX)  # [18, U, 512]
        for u, unit in enumerate(core_units[c]):
            for a, tile in enumerate(unit):
                m, yi, xi, keep = tile
                px = img[3 * a:3 * a + 3, u].reshape(3, TILE_R, TILE_C)
                image[yi * TILE_R:(yi + 1) * TILE_R,
                      xi * TILE_C:(xi + 1) * TILE_C] = px.transpose(1, 2, 0)
    return image


if __name__ == "__main__":
    import reference  # dev only
    inp = reference.setup_inputs()
    out = kernel(**{k: np.asarray(v) for k, v in inp.items()})
    print(out.shape, out.dtype)


# revision 15
# speedup vs baseline: 1.0667x; 1.0667x over previous
"""Trainium2 Bass kernel for the differentiable Gaussian renderer.

Strategy
--------
Host (numpy, 512 Gaussians, negligible):
  - mirror the reference projection exactly: quat->rot, 3D cov, camera
    transform, 2D cov (+eps fix), conic, visibility, back-to-front sort.
  - split the image into 96 tiles of 16 rows x 32 cols (512 px).  Per
    tile, cull Gaussians whose max alpha over the tile is < ALPHA_CUT
    (exact quadratic max over the tile rectangle; culling alone is
    ~2.6e-3 rel err vs the 2e-2 budget).  Per-tile recentered (x', y')
    coordinates keep fp32/f32r cancellation error small.
  - PARTITION-PACK up to 6 tiles into one 128-row "unit"
    (first-fit-decreasing; 24 units; 3 per core).  ACT cost on TRN2
    scales only with the free (pixel) dim, so packing tiles onto
    disjoint partition ranges of shared [128, 512px] tensors divides
    the scalar-engine work — the kernel bottleneck — by the pack factor.
  - TELESCOPED compositing: alpha_i*T_i = T_i - T_{i+1}, so
    img = sum_i (c_i - c_{i-1}) * T_i with T_i = exp(S_i),
    S_i = sum_{j<i} ln(1 - alpha_j) (strict cumsum within each tile
    segment).  The background becomes a final row with color bg and
    alpha irrelevant; no exp(E+S) in-place PSUM accumulation is needed.

Device (8 cores SPMD, 3 units each = 12 tiles, software-pipelined):
  Gaussians on partitions (depth order, tile segments stacked per
  unit), 512 tile-local pixels on the free dim (pixel features are
  identical for every tile after recentering).  Per unit:
  - PE : E = gc^T(6x128) @ pf(6x512)                  -> PSUM
  - ACT: A = exp(E)                                   -> SBUF
  - ACT: L = ln(1 - A)                                -> SBUF
  - PE : S = blocktriu_u^T @ L    (overwrites E bank) -> PSUM
  - ACT: T = exp(S)                                   -> SBUF
  - PE : img = dcol_u^T @ T  (rows 0:18 of the dead E/S bank; PE can
         only write PSUM at 32-aligned partition bases)
  - per-stage PSUM->SBUF copy + output DMA (last stage on ScalarE,
    whose HWDGE queue is idle, to shorten the pipeline drain).
  The activation-table monkeypatch pins the single exp+ln table set so
  the ACT table loader never reloads mid-kernel.
"""

import numpy as np

import concourse.bacc as bacc
import concourse.tile as _tile_mod
from concourse import hw_specs as _hw_specs, mybir
from concourse.bass_utils import run_bass_kernel_spmd


def _mono_act_tables(module_arch):
    """Blank every activation-table set except the one holding BOTH exp and
    ln, so the table loader never thrashes between Exp and Ln tables."""
    tables = _hw_specs.get_activation_tables(module_arch)
    keep = "natural_log_exp_and_others"
    if keep in tables:
        return {n: (s if n == keep else set()) for n, s in tables.items()}
    return tables


bacc.get_activation_tables = _mono_act_tables

H, W = 192, 256
NEAR, FAR = 0.1, 100.0
N = 512
N_CORES = 8
TILE_R, TILE_C = 16, 32
TILE_PX = TILE_R * TILE_C          # 512
N_TILES_Y = H // TILE_R            # 12
N_TILES_X = W // TILE_C            # 8
N_TILES = N_TILES_Y * N_TILES_X    # 96
ALPHA_CUT = 2e-2                   # cull-only rel err ~2.6e-3 (budget 2e-2)
F32 = mybir.dt.float32
F32R = mybir.dt.float32r

_TileContext = _tile_mod.TileContext


# ---------------------------------------------------------------- host math

def _project_and_sort(positions, scales, rotations, opacities, colors,
                      view_matrix, fov_x):
    f8 = np.float64
    pos = positions.astype(f8)
    scl = scales.astype(f8)
    rot = rotations.astype(f8)
    opa = opacities.astype(f8)
    col = colors.astype(f8)
    vm = view_matrix.astype(f8)

    qn = rot / np.linalg.norm(rot, axis=-1, keepdims=True)
    w, x, y, z = qn[:, 0], qn[:, 1], qn[:, 2], qn[:, 3]
    R = np.stack([
        1 - 2 * (y * y + z * z), 2 * (x * y - w * z), 2 * (x * z + w * y),
        2 * (x * y + w * z), 1 - 2 * (x * x + z * z), 2 * (y * z - w * x),
        2 * (x * z - w * y), 2 * (y * z + w * x), 1 - 2 * (x * x + y * y),
    ], axis=-1).reshape(-1, 3, 3)
    cov3d = np.einsum('nij,nj,nkj->nik', R, scl ** 2, R)

    fx = W / (2.0 * np.tan(np.deg2rad(float(fov_x)) / 2.0))
    Wr = vm[:3, :3]
    t = vm[:3, 3]
    cam = pos @ Wr.T + t[None, :]
    depths = cam[:, 2]
    zs = np.maximum(depths, NEAR)
    X, Y = cam[:, 0], cam[:, 1]
    mx = fx * X / zs + W / 2.0
    my = H / 2.0 - fx * Y / zs
    zero = np.zeros_like(zs)
    J = np.stack([
        np.stack([fx / zs, zero, -fx * X / zs ** 2], axis=-1),
        np.stack([zero, fx / zs, -fx * Y / zs ** 2], axis=-1),
    ], axis=1)
    T2 = np.einsum('nij,jk->nik', J, Wr)
    cov2d = np.einsum('nij,njk,nlk->nil', T2, cov3d, T2)
    cov2d = 0.5 * (cov2d + np.swapaxes(cov2d, 1, 2))

    a, b, c = cov2d[:, 0, 0], cov2d[:, 0, 1], cov2d[:, 1, 1]
    mean_e = 0.5 * (a + c)
    disc = np.sqrt(np.maximum(0.25 * (a - c) ** 2 + b ** 2, 0.0))
    min_eig = mean_e - disc
    eps = np.where(min_eig <= 0, np.abs(min_eig) + 1e-6, 0.0)
    a = a + eps
    c = c + eps
    max_eig = mean_e + eps + disc
    radii = np.ceil(3.0 * np.sqrt(np.maximum(max_eig, 1e-6)))

    visible = (depths > NEAR) & (depths < FAR) & (radii > 0)
    # float32 keys + stable sort reproduce jnp.argsort's order exactly
    key = np.where(visible, -depths.astype(np.float32), np.inf).astype(np.float32)
    order = np.argsort(key, kind='stable')

    a_s, b_s, c_s = a[order], b[order], c[order]
    det = np.maximum(a_s * c_s - b_s * b_s, 1e-12)
    vis = visible[order]
    ca = np.where(vis, c_s / det, 0.0)
    cb = np.where(vis, -b_s / det, 0.0)
    cc = np.where(vis, a_s / det, 0.0)
    op = 1.0 / (1.0 + np.exp(-opa[order, 0]))
    return dict(
        ca=ca, cb=cb, cc=cc,
        mx=np.where(vis, mx[order], 0.0), my=np.where(vis, my[order], 0.0),
        L0=np.where(vis, np.log(np.maximum(op, 1e-300)), -100.0),
        op=op, col=np.where(vis[:, None], col[order], 0.0), vis=vis,
    )


def _tile_max_E(g, yc, ye, xc, xe):
    """Per-Gaussian max of E over the pixel rectangle [xc+-xe] x [yc+-ye]."""
    ca, cb, cc = g['ca'], g['cb'], g['cc']
    mx, my, L0 = g['mx'], g['my'], g['L0']
    best = np.where((my >= yc - ye) & (my <= yc + ye)
                    & (mx >= xc - xe) & (mx <= xc + xe), L0, -np.inf)
    safe_ca = np.where(ca > 0, ca, 1.0)
    safe_cc = np.where(cc > 0, cc, 1.0)
    for yv in (yc - ye, yc + ye):
        dy = yv - my
        xstar = np.clip(np.where(ca > 0, mx - cb * dy / safe_ca, mx),
                        xc - xe, xc + xe)
        for xv in (xstar, np.full_like(xstar, xc - xe),
                   np.full_like(xstar, xc + xe)):
            dx = xv - mx
            E = -0.5 * (ca * dx * dx + 2 * cb * dx * dy + cc * dy * dy) + L0
            best = np.maximum(best, E)
    for xv in (xc - xe, xc + xe):
        dx = xv - mx
        ystar = np.clip(np.where(cc > 0, my - cb * dx / safe_cc, my),
                        yc - ye, yc + ye)
        dy = ystar - my
        E = -0.5 * (ca * dx * dx + 2 * cb * dx * dy + cc * dy * dy) + L0
        best = np.maximum(best, E)
    return np.where(g['vis'], best, -np.inf)


def _pixel_features():
    """[6, 512] recentered tile pixel features, row-major within the tile."""
    xs = np.arange(TILE_C, dtype=np.float64) - (TILE_C - 1) / 2.0
    ys = np.arange(TILE_R, dtype=np.float64) - (TILE_R - 1) / 2.0
    yy, xx = np.meshgrid(ys, xs, indexing='ij')
    xx = xx.ravel()
    yy = yy.ravel()
    feats = np.stack([xx * xx, yy * yy, xx * yy, xx, yy,
                      np.ones_like(xx)], axis=0)
    return feats.astype(np.float32)


def _segment_data(g, keep, yc, xc, bg_color):
    """E-coefficients [6, m], delta-colors [m, 3] for one tile segment.

    Rows = culled Gaussians in depth order, then one background row whose
    color closes the telescoped sum (its alpha is never used).
    """
    ca, cb, cc = g['ca'][keep], g['cb'][keep], g['cc'][keep]
    mxp = g['mx'][keep] - xc
    myp = g['my'][keep] - yc
    L0 = g['L0'][keep]
    n = len(keep)
    m = n + 1
    gc = np.empty((6, m), np.float64)
    gc[0, :n] = -0.5 * ca
    gc[1, :n] = -0.5 * cc
    gc[2, :n] = -cb
    gc[3, :n] = ca * mxp + cb * myp
    gc[4, :n] = cc * myp + cb * mxp
    gc[5, :n] = -0.5 * (ca * mxp ** 2 + 2 * cb * mxp * myp
                        + cc * myp ** 2) + L0
    gc[:, n] = 0.0
    gc[5, n] = -100.0                     # bg row: alpha ~ 0 (unused)
    cols = np.empty((m, 3), np.float64)
    cols[:n] = g['col'][keep]
    cols[n] = bg_color
    dcol = np.empty_like(cols)
    dcol[0] = cols[0]
    dcol[1:] = cols[1:] - cols[:-1]       # telescoped colors
    return gc.astype(np.float32), dcol.astype(np.float32)


# ------------------------------------------------------------- device program

_NC_CACHE = {}


def _stage_widths(n_units):
    """Pipeline stage widths: pairs, then two single-unit drain stages."""
    widths = []
    rem = int(n_units)
    while rem > 2:
        widths.append(2)
        rem -= 2
    widths.extend([1] * rem)
    return widths


def _build_nc(n_units):
    """Device program for n_units partition-packed units per core."""
    U = int(n_units)
    widths = _stage_widths(U)
    n_stages = len(widths)
    offs = np.concatenate([[0], np.cumsum(widths)]).astype(int)

    nc = bacc.Bacc()
    # gp = pixel features || per-unit E-coefficient blocks (6 partitions)
    gp_d = nc.dram_tensor("gp", [6, TILE_PX + U * 128], F32R,
                          kind="ExternalInput")
    # tcd = per-unit block-triu masks || per-unit delta-colors
    tcd_d = nc.dram_tensor("tcd", [128, U * 128 + U * 18], F32R,
                           kind="ExternalInput")
    img_d = nc.dram_tensor("img", [18, U * TILE_PX], F32,
                           kind="ExternalOutput")

    EXP = mybir.ActivationFunctionType.Exp
    LN = mybir.ActivationFunctionType.Ln

    with _TileContext(nc) as tc:
        with (
            tc.tile_pool(name="consts", bufs=1) as consts,
            tc.tile_pool(name="abuf", bufs=2) as apool,
            tc.tile_pool(name="lbuf", bufs=2) as lpool,
            tc.tile_pool(name="tbuf", bufs=2) as tpool,
            tc.tile_pool(name="obuf", bufs=3) as obuf,
            tc.tile_pool(name="espsum", bufs=min(n_stages, 4),
                         space="PSUM") as epool,
        ):
            gp_sb = consts.tile([6, TILE_PX + U * 128], F32R)
            # first piece only carries what gates the first E matmuls;
            # ScalarE's HWDGE queue is free before the table load, so it
            # issues ~1us earlier than SP (which still has preamble work)
            w0 = TILE_PX + widths[0] * 128
            nc.scalar.dma_start(out=gp_sb[:, 0:w0], in_=gp_d[:, 0:w0])
            nc.sync.dma_start(out=gp_sb[:, w0:], in_=gp_d[:, w0:])
            tcd_sb = consts.tile([128, U * 128 + U * 18], F32R)
            nc.sync.dma_start(out=tcd_sb, in_=tcd_d[:])

            pf = gp_sb[:, 0:TILE_PX]
            gc_sb = gp_sb[:, TILE_PX:].rearrange("p (u g) -> p u g", g=128)
            triu_sb = tcd_sb[:, 0:U * 128].rearrange(
                "p (u g) -> p u g", g=128)
            dcol_sb = tcd_sb[:, U * 128:].rearrange(
                "p (u s) -> p u s", s=18)

            es_tiles = {}

            def emit_e(s):
                es = epool.tile([128, widths[s], TILE_PX], F32, tag="es")
                es_tiles[s] = es
                for j in range(widths[s]):
                    nc.tensor.matmul(es[:, j, :], gc_sb[:, offs[s] + j, :],
                                     pf, start=True, stop=True)

            def act(pool, dt, func, src, s, **kw):
                t = pool.tile([128, widths[s], TILE_PX], dt, tag=pool.name)
                nc.scalar.activation(
                    t.rearrange("p a b -> p (a b)"),
                    src.rearrange("p a b -> p (a b)"), func, **kw)
                return t

            # software pipeline over stages; ScalarE queue is strict FIFO;
            # intended ACT order: exp0 ln0 exp1 ln1 T0 exp2 ln2 T1 ...
            a_t = {}
            l_t = {}
            emit_e(0)
            a_t[0] = act(apool, F32, EXP, es_tiles[0], 0)
            l_t[0] = act(lpool, F32R, LN, a_t[0], 0, bias=1.0, scale=-1.0)
            for s in range(n_stages):
                es = es_tiles[s]
                # S = blocktriu^T @ L overwrites the E bank (E fully
                # consumed by exp); separate accumulation group.
                for j in range(widths[s]):
                    nc.tensor.matmul(es[:, j, :],
                                     triu_sb[:, offs[s] + j, :],
                                     l_t[s][:, j, :], start=True, stop=True)
                if s + 1 < n_stages:
                    emit_e(s + 1)
                    a_t[s + 1] = act(apool, F32, EXP, es_tiles[s + 1], s + 1)
                    l_t[s + 1] = act(lpool, F32R, LN, a_t[s + 1], s + 1,
                                     bias=1.0, scale=-1.0)
                t_t = act(tpool, F32R, EXP, es, s)
                for j in range(widths[s]):
                    u = offs[s] + j
                    # the stage's own E/S bank is dead after the T exp read;
                    # park the unit's 18-row image in its rows 0:18 (PE can
                    # only write PSUM at 32-aligned partition bases)
                    nc.tensor.matmul(es[0:18, j, :],
                                     dcol_sb[:, u, :], t_t[:, j, :],
                                     start=True, stop=True)
                ob = obuf.tile([18, widths[s], TILE_PX], F32, tag="ob")
                if s == n_stages - 1:
                    # ScalarE is idle after the final T exp; DVE may still
                    # be draining the previous stage's wider copy.  Its
                    # HWDGE queue is also free, unlike SP which may still
                    # be issuing the previous stage's output DMA.
                    nc.scalar.copy(ob.rearrange("p a b -> p (a b)"),
                                   es[0:18, :, :].rearrange("p a b -> p (a b)"))
                    nc.scalar.dma_start(
                        out=img_d[:, offs[s] * TILE_PX:offs[s + 1] * TILE_PX],
                        in_=ob.rearrange("p a b -> p (a b)"))
                else:
                    nc.vector.tensor_copy(ob, es[0:18, :, :])
                    nc.sync.dma_start(
                        out=img_d[:, offs[s] * TILE_PX:offs[s + 1] * TILE_PX],
                        in_=ob.rearrange("p a b -> p (a b)"))
    nc.finalize()
    return nc


def _get_nc(n_units):
    key = int(n_units)
    if key not in _NC_CACHE:
        _NC_CACHE[key] = _build_nc(key)
    return _NC_CACHE[key]


# ----------------------------------------------------------------- entrypoint

def kernel(positions, scales, rotations, opacities, colors, view_matrix,
           background, fov_x):
    g = _project_and_sort(positions, scales, rotations, opacities, colors,
                          view_matrix, fov_x)
    assert g['op'][g['vis']].max() < 0.985, "alpha clip at 0.99 would activate"
    bg = np.asarray(background, np.float64)

    cut = float(np.log(ALPHA_CUT))
    xe = (TILE_C - 1) / 2.0
    ye = (TILE_R - 1) / 2.0

    tiles = []                       # (m, yi, xi, keep); m = rows incl. bg
    for yi in range(N_TILES_Y):
        yc = yi * TILE_R + ye
        for xi in range(N_TILES_X):
            xc = xi * TILE_C + xe
            keep = np.nonzero(_tile_max_E(g, yc, ye, xc, xe) >= cut)[0]
            assert len(keep) + 1 <= 128, "tile exceeds one partition unit"
            tiles.append((len(keep) + 1, yi, xi, keep))

    # First-fit-decreasing: pack tiles into 128-row units, <=3 tiles each.
    tiles.sort(key=lambda t: -t[0])
    units = []                       # list of lists of tiles
    for t in tiles:
        for u in units:
            if sum(x[0] for x in u) + t[0] <= 128 and len(u) < 6:
                u.append(t)
                break
        else:
            units.append([t])
    # Deal units round-robin to cores; every unit costs the same on device.
    core_units = [units[c::N_CORES] for c in range(N_CORES)]
    U = max(len(cu) for cu in core_units)

    pf = _pixel_features()

    in_maps = []
    for c in range(N_CORES):
        gc_dev = np.zeros((6, U, 128), np.float32)
        gc_dev[5] = -100.0                           # padding: alpha ~ 0
        triu_dev = np.zeros((128, U, 128), np.float32)
        dcol_dev = np.zeros((128, U, 18), np.float32)
        for u, unit in enumerate(core_units[c]):
            r = 0
            for a, tile in enumerate(unit):
                m, yi, xi, keep = tile
                yc = yi * TILE_R + ye
                xc = xi * TILE_C + xe
                gc_t, dcol_t = _segment_data(g, keep, yc, xc, bg)
                gc_dev[:, u, r:r + m] = gc_t
                dcol_dev[r:r + m, u, 3 * a:3 * a + 3] = dcol_t
                triu_dev[r:r + m, u, r:r + m] = np.triu(
                    np.ones((m, m), np.float32), 1)
                r += m
        gp = np.concatenate(
            [pf, gc_dev.reshape(6, U * 128)], axis=1)
        tcd = np.concatenate(
            [triu_dev.reshape(128, U * 128),
             dcol_dev.reshape(128, U * 18)], axis=1)
        in_maps.append(dict(gp=np.ascontiguousarray(gp),
                            tcd=np.ascontiguousarray(tcd)))

    res = run_bass_kernel_spmd(_get_nc(U), in_maps,
                               core_ids=list(range(N_CORES)))

    image = np.empty((H, W, 3), np.float32)
    for c in range(N_CORES):
        img = res.results[c]["img"].reshape(18, -1, TILE_PX)  # [18, U, 512]
        for u, unit in enumerate(core_units[c]):
            for a, tile in enumerate(unit):
                m, yi, xi, keep = tile
                px = img[3 * a:3 * a + 3, u].reshape(3, TILE_R, TILE_C)
                image[yi * TILE_R:(yi + 1) * TILE_R,
                      xi * TILE_C:(xi + 1) * TILE_C] = px.transpose(1, 2, 0)
    return image


if __name__ == "__main__":
    import reference  # dev only
    inp = reference.setup_inputs()
    out = kernel(**{k: np.asarray(v) for k, v in inp.items()})
    print(out.shape, out.dtype)


# revision 17
# speedup vs baseline: 1.1063x; 1.0372x over previous
"""Trainium2 Bass kernel for the differentiable Gaussian renderer.

Strategy
--------
Host (numpy, 512 Gaussians, negligible):
  - mirror the reference projection exactly: quat->rot, 3D cov, camera
    transform, 2D cov (+eps fix), conic, visibility, back-to-front sort.
  - split the image into 96 tiles of 16 rows x 32 cols (512 px).  Per
    tile, cull Gaussians whose max alpha over the tile is < ALPHA_CUT
    (exact quadratic max over the tile rectangle; culling alone is
    ~2.6e-3 rel err vs the 2e-2 budget).  Per-tile recentered (x', y')
    coordinates keep fp32/f32r cancellation error small.
  - PARTITION-PACK up to 6 tiles into one 128-row "unit"
    (first-fit-decreasing; 24 units; 3 per core).  ACT cost on TRN2
    scales only with the free (pixel) dim, so packing tiles onto
    disjoint partition ranges of shared [128, 512px] tensors divides
    the scalar-engine work — the kernel bottleneck — by the pack factor.
  - TELESCOPED compositing: alpha_i*T_i = T_i - T_{i+1}, so
    img = sum_i (c_i - c_{i-1}) * T_i with T_i = exp(S_i),
    S_i = sum_{j<i} ln(1 - alpha_j) (strict cumsum within each tile
    segment).  The background becomes a final row with color bg and
    alpha irrelevant; no exp(E+S) in-place PSUM accumulation is needed.

Device (8 cores SPMD, 3 units each = 12 tiles, software-pipelined):
  Gaussians on partitions (depth order, tile segments stacked per
  unit), 512 tile-local pixels on the free dim (pixel features are
  identical for every tile after recentering).  Per unit:
  - PE : E = gc^T(6x128) @ pf(6x512)                  -> PSUM
  - ACT: A = exp(E)                                   -> SBUF
  - ACT: L = ln(1 - A)                                -> SBUF
  - PE : S = blocktriu_u^T @ L    (overwrites E bank) -> PSUM
  - ACT: T = exp(S)                                   -> SBUF
  - PE : img = dcol_u^T @ T  (rows 0:18 of the dead E/S bank; PE can
         only write PSUM at 32-aligned partition bases)
  - per-stage PSUM->SBUF copy + output DMA (last stage on ScalarE,
    whose HWDGE queue is idle, to shorten the pipeline drain).
  The activation-table monkeypatch pins the single exp+ln table set so
  the ACT table loader never reloads mid-kernel.
"""

import numpy as np

import concourse.bacc as bacc
import concourse.tile as _tile_mod
from concourse import hw_specs as _hw_specs, mybir
from concourse.bass_utils import run_bass_kernel_spmd


def _mono_act_tables(module_arch):
    """Blank every activation-table set except the one holding BOTH exp and
    ln, so the table loader never thrashes between Exp and Ln tables."""
    tables = _hw_specs.get_activation_tables(module_arch)
    keep = "natural_log_exp_and_others"
    if keep in tables:
        return {n: (s if n == keep else set()) for n, s in tables.items()}
    return tables


bacc.get_activation_tables = _mono_act_tables

H, W = 192, 256
NEAR, FAR = 0.1, 100.0
N = 512
N_CORES = 8
TILE_R, TILE_C = 16, 32
TILE_PX = TILE_R * TILE_C          # 512
N_TILES_Y = H // TILE_R            # 12
N_TILES_X = W // TILE_C            # 8
N_TILES = N_TILES_Y * N_TILES_X    # 96
ALPHA_CUT = 2e-2                   # cull-only rel err ~2.6e-3 (budget 2e-2)
F32 = mybir.dt.float32
F32R = mybir.dt.float32r

_TileContext = _tile_mod.TileContext


# ---------------------------------------------------------------- host math

def _project_and_sort(positions, scales, rotations, opacities, colors,
                      view_matrix, fov_x):
    f8 = np.float64
    pos = positions.astype(f8)
    scl = scales.astype(f8)
    rot = rotations.astype(f8)
    opa = opacities.astype(f8)
    col = colors.astype(f8)
    vm = view_matrix.astype(f8)

    qn = rot / np.linalg.norm(rot, axis=-1, keepdims=True)
    w, x, y, z = qn[:, 0], qn[:, 1], qn[:, 2], qn[:, 3]
    R = np.stack([
        1 - 2 * (y * y + z * z), 2 * (x * y - w * z), 2 * (x * z + w * y),
        2 * (x * y + w * z), 1 - 2 * (x * x + z * z), 2 * (y * z - w * x),
        2 * (x * z - w * y), 2 * (y * z + w * x), 1 - 2 * (x * x + y * y),
    ], axis=-1).reshape(-1, 3, 3)
    cov3d = np.einsum('nij,nj,nkj->nik', R, scl ** 2, R)

    fx = W / (2.0 * np.tan(np.deg2rad(float(fov_x)) / 2.0))
    Wr = vm[:3, :3]
    t = vm[:3, 3]
    cam = pos @ Wr.T + t[None, :]
    depths = cam[:, 2]
    zs = np.maximum(depths, NEAR)
    X, Y = cam[:, 0], cam[:, 1]
    mx = fx * X / zs + W / 2.0
    my = H / 2.0 - fx * Y / zs
    zero = np.zeros_like(zs)
    J = np.stack([
        np.stack([fx / zs, zero, -fx * X / zs ** 2], axis=-1),
        np.stack([zero, fx / zs, -fx * Y / zs ** 2], axis=-1),
    ], axis=1)
    T2 = np.einsum('nij,jk->nik', J, Wr)
    cov2d = np.einsum('nij,njk,nlk->nil', T2, cov3d, T2)
    cov2d = 0.5 * (cov2d + np.swapaxes(cov2d, 1, 2))

    a, b, c = cov2d[:, 0, 0], cov2d[:, 0, 1], cov2d[:, 1, 1]
    mean_e = 0.5 * (a + c)
    disc = np.sqrt(np.maximum(0.25 * (a - c) ** 2 + b ** 2, 0.0))
    min_eig = mean_e - disc
    eps = np.where(min_eig <= 0, np.abs(min_eig) + 1e-6, 0.0)
    a = a + eps
    c = c + eps
    max_eig = mean_e + eps + disc
    radii = np.ceil(3.0 * np.sqrt(np.maximum(max_eig, 1e-6)))

    visible = (depths > NEAR) & (depths < FAR) & (radii > 0)
    # float32 keys + stable sort reproduce jnp.argsort's order exactly
    key = np.where(visible, -depths.astype(np.float32), np.inf).astype(np.float32)
    order = np.argsort(key, kind='stable')

    a_s, b_s, c_s = a[order], b[order], c[order]
    det = np.maximum(a_s * c_s - b_s * b_s, 1e-12)
    vis = visible[order]
    ca = np.where(vis, c_s / det, 0.0)
    cb = np.where(vis, -b_s / det, 0.0)
    cc = np.where(vis, a_s / det, 0.0)
    op = 1.0 / (1.0 + np.exp(-opa[order, 0]))
    return dict(
        ca=ca, cb=cb, cc=cc,
        mx=np.where(vis, mx[order], 0.0), my=np.where(vis, my[order], 0.0),
        L0=np.where(vis, np.log(np.maximum(op, 1e-300)), -100.0),
        op=op, col=np.where(vis[:, None], col[order], 0.0), vis=vis,
    )


def _tile_max_E(g, yc, ye, xc, xe):
    """Per-Gaussian max of E over the pixel rectangle [xc+-xe] x [yc+-ye]."""
    ca, cb, cc = g['ca'], g['cb'], g['cc']
    mx, my, L0 = g['mx'], g['my'], g['L0']
    best = np.where((my >= yc - ye) & (my <= yc + ye)
                    & (mx >= xc - xe) & (mx <= xc + xe), L0, -np.inf)
    safe_ca = np.where(ca > 0, ca, 1.0)
    safe_cc = np.where(cc > 0, cc, 1.0)
    for yv in (yc - ye, yc + ye):
        dy = yv - my
        xstar = np.clip(np.where(ca > 0, mx - cb * dy / safe_ca, mx),
                        xc - xe, xc + xe)
        for xv in (xstar, np.full_like(xstar, xc - xe),
                   np.full_like(xstar, xc + xe)):
            dx = xv - mx
            E = -0.5 * (ca * dx * dx + 2 * cb * dx * dy + cc * dy * dy) + L0
            best = np.maximum(best, E)
    for xv in (xc - xe, xc + xe):
        dx = xv - mx
        ystar = np.clip(np.where(cc > 0, my - cb * dx / safe_cc, my),
                        yc - ye, yc + ye)
        dy = ystar - my
        E = -0.5 * (ca * dx * dx + 2 * cb * dx * dy + cc * dy * dy) + L0
        best = np.maximum(best, E)
    return np.where(g['vis'], best, -np.inf)


def _pixel_features():
    """[6, 512] recentered tile pixel features, row-major within the tile."""
    xs = np.arange(TILE_C, dtype=np.float64) - (TILE_C - 1) / 2.0
    ys = np.arange(TILE_R, dtype=np.float64) - (TILE_R - 1) / 2.0
    yy, xx = np.meshgrid(ys, xs, indexing='ij')
    xx = xx.ravel()
    yy = yy.ravel()
    feats = np.stack([xx * xx, yy * yy, xx * yy, xx, yy,
                      np.ones_like(xx)], axis=0)
    return feats.astype(np.float32)


def _segment_data(g, keep, yc, xc, bg_color):
    """E-coefficients [6, m], delta-colors [m, 3] for one tile segment.

    Rows = culled Gaussians in depth order, then one background row whose
    color closes the telescoped sum (its alpha is never used).
    """
    ca, cb, cc = g['ca'][keep], g['cb'][keep], g['cc'][keep]
    mxp = g['mx'][keep] - xc
    myp = g['my'][keep] - yc
    L0 = g['L0'][keep]
    n = len(keep)
    m = n + 1
    gc = np.empty((6, m), np.float64)
    gc[0, :n] = -0.5 * ca
    gc[1, :n] = -0.5 * cc
    gc[2, :n] = -cb
    gc[3, :n] = ca * mxp + cb * myp
    gc[4, :n] = cc * myp + cb * mxp
    gc[5, :n] = -0.5 * (ca * mxp ** 2 + 2 * cb * mxp * myp
                        + cc * myp ** 2) + L0
    gc[:, n] = 0.0
    gc[5, n] = -100.0                     # bg row: alpha ~ 0 (unused)
    cols = np.empty((m, 3), np.float64)
    cols[:n] = g['col'][keep]
    cols[n] = bg_color
    dcol = np.empty_like(cols)
    dcol[0] = cols[0]
    dcol[1:] = cols[1:] - cols[:-1]       # telescoped colors
    return gc.astype(np.float32), dcol.astype(np.float32)


# ------------------------------------------------------------- device program

_NC_CACHE = {}


def _stage_widths(n_units):
    """Pipeline stage widths: pairs, then two single-unit drain stages."""
    widths = []
    rem = int(n_units)
    while rem > 2:
        widths.append(2)
        rem -= 2
    widths.extend([1] * rem)
    return widths


def _build_nc(n_units):
    """Device program for n_units partition-packed units per core."""
    U = int(n_units)
    widths = _stage_widths(U)
    n_stages = len(widths)
    offs = np.concatenate([[0], np.cumsum(widths)]).astype(int)

    nc = bacc.Bacc()
    # gp = pixel features || per-unit E-coefficient blocks (6 partitions)
    gp_d = nc.dram_tensor("gp", [6, TILE_PX + U * 128], F32R,
                          kind="ExternalInput")
    # tcd = per-unit block-triu masks || per-unit delta-colors
    tcd_d = nc.dram_tensor("tcd", [128, U * 128 + U * 18], F32R,
                           kind="ExternalInput")
    img_d = nc.dram_tensor("img", [18, U * TILE_PX], F32,
                           kind="ExternalOutput")

    EXP = mybir.ActivationFunctionType.Exp
    LN = mybir.ActivationFunctionType.Ln

    with _TileContext(nc) as tc:
        with (
            tc.tile_pool(name="consts", bufs=1) as consts,
            tc.tile_pool(name="abuf", bufs=2) as apool,
            tc.tile_pool(name="lbuf", bufs=2) as lpool,
            tc.tile_pool(name="tbuf", bufs=2) as tpool,
            tc.tile_pool(name="obuf", bufs=3) as obuf,
            tc.tile_pool(name="espsum", bufs=min(n_stages, 4),
                         space="PSUM") as epool,
            tc.tile_pool(name="jpsum", bufs=1, space="PSUM") as jpool,
        ):
            gp_sb = consts.tile([6, TILE_PX + U * 128], F32R)
            # first piece only carries what gates the first E matmuls;
            # ScalarE's HWDGE queue is free before the table load, so it
            # issues ~1us earlier than SP (which still has preamble work)
            w0 = TILE_PX + widths[0] * 128
            nc.scalar.dma_start(out=gp_sb[:, 0:w0], in_=gp_d[:, 0:w0])
            nc.sync.dma_start(out=gp_sb[:, w0:], in_=gp_d[:, w0:])
            tcd_sb = consts.tile([128, U * 128 + U * 18], F32R)
            nc.sync.dma_start(out=tcd_sb, in_=tcd_d[:])

            # HAM warm-up: the PE clock gate sits at 4/8 (1.2 GHz) until
            # ~3.4us of sustained matmul activity.  The input DMA latency
            # leaves the PE idle for ~3.8us at kernel start — fill it with
            # junk matmuls on a zeroed scratch tile so every real matmul
            # (and the ln->S->T critical chains) runs at 8/8 (2.4 GHz).
            scr = consts.tile([128, 192], F32, name="warm_src")
            nc.gpsimd.memset(scr, 0.0)
            scr_r = scr.bitcast(F32R)
            jnk = jpool.tile([128, 64], F32, name="warm_out")
            for _ in range(10):
                nc.tensor.matmul(jnk, scr_r[:, 0:128], scr_r[:, 128:192],
                                 start=True, stop=True)

            pf = gp_sb[:, 0:TILE_PX]
            gc_sb = gp_sb[:, TILE_PX:].rearrange("p (u g) -> p u g", g=128)
            triu_sb = tcd_sb[:, 0:U * 128].rearrange(
                "p (u g) -> p u g", g=128)
            dcol_sb = tcd_sb[:, U * 128:].rearrange(
                "p (u s) -> p u s", s=18)

            es_tiles = {}

            def emit_e(s):
                es = epool.tile([128, widths[s], TILE_PX], F32, tag="es")
                es_tiles[s] = es
                for j in range(widths[s]):
                    nc.tensor.matmul(es[:, j, :], gc_sb[:, offs[s] + j, :],
                                     pf, start=True, stop=True)

            def act(pool, dt, func, src, s, **kw):
                t = pool.tile([128, widths[s], TILE_PX], dt, tag=pool.name)
                nc.scalar.activation(
                    t.rearrange("p a b -> p (a b)"),
                    src.rearrange("p a b -> p (a b)"), func, **kw)
                return t

            # software pipeline over stages; ScalarE queue is strict FIFO;
            # intended ACT order: exp0 ln0 exp1 ln1 T0 exp2 ln2 T1 ...
            a_t = {}
            l_t = {}
            emit_e(0)
            a_t[0] = act(apool, F32, EXP, es_tiles[0], 0)
            l_t[0] = act(lpool, F32R, LN, a_t[0], 0, bias=1.0, scale=-1.0)
            for s in range(n_stages):
                es = es_tiles[s]
                # S = blocktriu^T @ L overwrites the E bank (E fully
                # consumed by exp); separate accumulation group.
                for j in range(widths[s]):
                    nc.tensor.matmul(es[:, j, :],
                                     triu_sb[:, offs[s] + j, :],
                                     l_t[s][:, j, :], start=True, stop=True)
                if s + 1 < n_stages:
                    emit_e(s + 1)
                    a_t[s + 1] = act(apool, F32, EXP, es_tiles[s + 1], s + 1)
                    l_t[s + 1] = act(lpool, F32R, LN, a_t[s + 1], s + 1,
                                     bias=1.0, scale=-1.0)
                t_t = act(tpool, F32R, EXP, es, s)
                for j in range(widths[s]):
                    u = offs[s] + j
                    # the stage's own E/S bank is dead after the T exp read;
                    # park the unit's 18-row image in its rows 0:18 (PE can
                    # only write PSUM at 32-aligned partition bases)
                    nc.tensor.matmul(es[0:18, j, :],
                                     dcol_sb[:, u, :], t_t[:, j, :],
                                     start=True, stop=True)
                ob = obuf.tile([18, widths[s], TILE_PX], F32, tag="ob")
                if s == n_stages - 1:
                    # ScalarE is idle after the final T exp; DVE may still
                    # be draining the previous stage's wider copy.  Its
                    # HWDGE queue is also free, unlike SP which may still
                    # be issuing the previous stage's output DMA.
                    nc.scalar.copy(ob.rearrange("p a b -> p (a b)"),
                                   es[0:18, :, :].rearrange("p a b -> p (a b)"))
                    nc.scalar.dma_start(
                        out=img_d[:, offs[s] * TILE_PX:offs[s + 1] * TILE_PX],
                        in_=ob.rearrange("p a b -> p (a b)"))
                else:
                    nc.vector.tensor_copy(ob, es[0:18, :, :])
                    nc.sync.dma_start(
                        out=img_d[:, offs[s] * TILE_PX:offs[s + 1] * TILE_PX],
                        in_=ob.rearrange("p a b -> p (a b)"))
    nc.finalize()
    return nc


def _get_nc(n_units):
    key = int(n_units)
    if key not in _NC_CACHE:
        _NC_CACHE[key] = _build_nc(key)
    return _NC_CACHE[key]


# ----------------------------------------------------------------- entrypoint

def kernel(positions, scales, rotations, opacities, colors, view_matrix,
           background, fov_x):
    g = _project_and_sort(positions, scales, rotations, opacities, colors,
                          view_matrix, fov_x)
    assert g['op'][g['vis']].max() < 0.985, "alpha clip at 0.99 would activate"
    bg = np.asarray(background, np.float64)

    cut = float(np.log(ALPHA_CUT))
    xe = (TILE_C - 1) / 2.0
    ye = (TILE_R - 1) / 2.0

    tiles = []                       # (m, yi, xi, keep); m = rows incl. bg
    for yi in range(N_TILES_Y):
        yc = yi * TILE_R + ye
        for xi in range(N_TILES_X):
            xc = xi * TILE_C + xe
            keep = np.nonzero(_tile_max_E(g, yc, ye, xc, xe) >= cut)[0]
            assert len(keep) + 1 <= 128, "tile exceeds one partition unit"
            tiles.append((len(keep) + 1, yi, xi, keep))

    # First-fit-decreasing: pack tiles into 128-row units, <=3 tiles each.
    tiles.sort(key=lambda t: -t[0])
    units = []                       # list of lists of tiles
    for t in tiles:
        for u in units:
            if sum(x[0] for x in u) + t[0] <= 128 and len(u) < 6:
                u.append(t)
                break
        else:
            units.append([t])
    # Deal units round-robin to cores; every unit costs the same on device.
    core_units = [units[c::N_CORES] for c in range(N_CORES)]
    U = max(len(cu) for cu in core_units)

    pf = _pixel_features()

    in_maps = []
    for c in range(N_CORES):
        gc_dev = np.zeros((6, U, 128), np.float32)
        gc_dev[5] = -100.0                           # padding: alpha ~ 0
        triu_dev = np.zeros((128, U, 128), np.float32)
        dcol_dev = np.zeros((128, U, 18), np.float32)
        for u, unit in enumerate(core_units[c]):
            r = 0
            for a, tile in enumerate(unit):
                m, yi, xi, keep = tile
                yc = yi * TILE_R + ye
                xc = xi * TILE_C + xe
                gc_t, dcol_t = _segment_data(g, keep, yc, xc, bg)
                gc_dev[:, u, r:r + m] = gc_t
                dcol_dev[r:r + m, u, 3 * a:3 * a + 3] = dcol_t
                triu_dev[r:r + m, u, r:r + m] = np.triu(
                    np.ones((m, m), np.float32), 1)
                r += m
        gp = np.concatenate(
            [pf, gc_dev.reshape(6, U * 128)], axis=1)
        tcd = np.concatenate(
            [triu_dev.reshape(128, U * 128),
             dcol_dev.reshape(128, U * 18)], axis=1)
        in_maps.append(dict(gp=np.ascontiguousarray(gp),
                            tcd=np.ascontiguousarray(tcd)))

    res = run_bass_kernel_spmd(_get_nc(U), in_maps,
                               core_ids=list(range(N_CORES)))

    image = np.empty((H, W, 3), np.float32)
    for c in range(N_CORES):
        img = res.results[c]["img"].reshape(18, -1, TILE_PX)  # [18, U, 512]
        for u, unit in enumerate(core_units[c]):
            for a, tile in enumerate(unit):
                m, yi, xi, keep = tile
                px = img[3 * a:3 * a + 3, u].reshape(3, TILE_R, TILE_C)
                image[yi * TILE_R:(yi + 1) * TILE_R,
                      xi * TILE_C:(xi + 1) * TILE_C] = px.transpose(1, 2, 0)
    return image


if __name__ == "__main__":
    import reference  # dev only
    inp = reference.setup_inputs()
    out = kernel(**{k: np.asarray(v) for k, v in inp.items()})
    print(out.shape, out.dtype)
